# revision 8
# baseline (speedup 1.0000x reference)
"""Trainium2 Bass kernel for nn_Choquet_integral (N_IN=12, N_OUT=16, M=16384).

Math (per input row x[0:12], fuzzy-measure table FM[4095, 16]):
    reference: sort x descending -> s, diffs_j = s_j - s_{j+1} (s_12 = 0),
    cumulative-set index c_j = sum_{t<=j} 2^{sortInd_t} - 1,
    out = sum_j diffs_j * FM[c_j].

v5: ranked-prefix (c_j) formulation -- 11 table lookups per row instead of
v3's 24 (A/B Abel form).  With T[v] = FM[v-1], T[0] = 0 ([4096, 16]):

    rank_i = #(j : x_j >=~ x_i)     (total order via 1-ulp bf16 nudge, i<j)
    R_j[i] = [rank_i <= j]          (top-j membership, j = 1..11)
    c_j    = sum_i 2^i R_j[i]       (top-j set mask; c_12 = 4095, T = 1)
    Q_j    = sum_i x_i R_j[i]       (sum of top-j values; Q_0 = 0)
    d_j    = 2Q_j - Q_{j-1} - Q_{j+1} = s_j - s_{j+1}
    y      = sum_{j=1..11} d_j T[c_j] + (Q_12 - Q_11)

Per core (2048 rows), layout partition p = 16c+q <-> row c*256+g*16+q:
  1. D = is_ge comparisons (v3's comparand tiles, 2 i-chunks x [128,1152]).
  2. rank via a weight-1 pairwise tree over the j dim (4 ops/chunk).
  3. R-stack [128, 11*192] via 11 tensor_scalar is_le ops.
  4. c_j via the 2^i radix tree over i (4 strided stt ops, int16 exact)
     -> c_int [128, 11*16] which IS the ap_gather index tile (col (j,g),
     partition-slot q: gather's 16-partition index sharing o-replicates).
  5. Q_j via x-weighted tree in f32 (bf16 would lose ~0.05 abs on Q~6).
  6. d -> dd [128, 192] bf16 (blocks d_1..d_11, s12).
  7. dd must reach the combine o-replicated: PE-transpose dd -> [192, 128]
     (2 matmul transposes), DMA to a DRAM scratch [192, 128], then 16
     partition-strided reads dr[16c+o, (jb*16+g)*16+q] <- dsc[z, c*16+q]
     (descriptors are 32B q-runs, contiguous both sides).
  8. 3 ap_gather calls (j-blocks 5+5+1; small last chunk shortens the
     post-gather tail) from the f32 column table
     t[16c+o] = T[:, o]: 2816 idx/DSP-core total vs v3's 6144 -- the Q7
     request loop is ~27ns/idx and dominates the kernel.
  9. combine V = G * dr (bf16), PE: 12 identity matmuls (11 gathered j's
     + the s12 block) accumulate yacc [128, 256]; copy out.
"""

import numpy as np
import ml_dtypes

import concourse.bacc as bacc
import concourse.bass as bass
import concourse.mybir as mybir
from concourse import library_config
from concourse.bass_utils import run_bass_kernel_spmd
from concourse.tile import TileContext

N_IN = 12
N_OUT = 16
N_VARS = 2**N_IN - 2  # 4094
M_FULL = 16384
N_CORES = 8
M_CORE = M_FULL // N_CORES  # 2048
NE = 2**N_IN  # 4096 table entries
NJ = 11  # gathered j-levels
F32 = mybir.dt.float32
BF16 = mybir.dt.bfloat16
I16 = mybir.dt.int16
BF = ml_dtypes.bfloat16


def _lattice_levels(n_in):
    levels = []
    for k in range(2, n_in):
        nodes = [s for s in range(1, 2**n_in - 1) if bin(s).count("1") == k]
        children = [
            [(s - (1 << b)) - 1 for b in range(n_in) if (s >> b) & 1] for s in nodes
        ]
        levels.append((np.array(nodes) - 1, np.array(children)))
    return levels


_LEVELS = _lattice_levels(N_IN)
_SINGLETONS = np.array([2**i - 1 for i in range(N_IN)])


def _build_table(fm_vars: np.ndarray) -> np.ndarray:
    """T_ext [4096, 16]: T_ext[0] = 0, T_ext[v] = FM[v-1]."""
    av = np.abs(fm_vars.astype(np.float32))
    FM = np.zeros((N_VARS, N_OUT), np.float32)
    FM[_SINGLETONS] = av[_SINGLETONS]
    for nodes_idx, children_idx in _LEVELS:
        mx = FM[children_idx].max(axis=1)
        FM[nodes_idx] = mx + av[nodes_idx]
    FM = np.concatenate([FM, np.ones((1, N_OUT), np.float32)], axis=0)
    FM = np.minimum(FM, np.float32(1.0))
    return np.concatenate([np.zeros((1, N_OUT), np.float32), FM], axis=0)


def build_bass(m_core: int = M_CORE, repeat: int = 1) -> bass.Bass:
    assert m_core % 2048 == 0
    G = m_core // 128  # 16 row-groups per partition
    n = N_IN
    ncol = n * G  # 192 (i-major: col = i*G + g)
    nj = n * ncol  # 2304
    nh = nj // 2  # 1152 per i-chunk (j 12 x i-half 6 x g 16)
    nr = NJ * ncol  # 2112 R-stack cols (j 1..11, i, g)
    nzz = (NJ + 1) * G  # 192 dd cols (jb 0..11, g)
    nc_ = G * 16  # 256 output cols (g*16+q)
    ngt = NJ * nc_  # 2816 gathered cols
    nc = bacc.Bacc()

    add = mybir.AluOpType.add
    mult = mybir.AluOpType.mult
    sub = mybir.AluOpType.subtract

    xza_d = nc.declare_dram_parameter("xza", [128, nh], BF16, isOutput=False)
    xzb_d = nc.declare_dram_parameter("xzb", [128, nh], BF16, isOutput=False)
    xea_d = nc.declare_dram_parameter("xea", [128, nh], BF16, isOutput=False)
    xeb_d = nc.declare_dram_parameter("xeb", [128, nh], BF16, isOutput=False)
    xg_d = nc.declare_dram_parameter("xg", [128, ncol], F32, isOutput=False)
    t_d = nc.declare_dram_parameter("t", [128, NE], F32, isOutput=False)
    id_d = nc.declare_dram_parameter("ident", [128, 128], BF16, isOutput=False)
    t1c_d = nc.declare_dram_parameter("t1c", [128, N_IN], BF16, isOutput=False)
    t11c_d = nc.declare_dram_parameter("t11c", [128, N_IN], BF16, isOutput=False)
    y_d = nc.declare_dram_parameter("y", [128, nc_], F32, isOutput=True)

    nzz2 = 9 * G + G + 2 * N_IN * G  # 544: d2..d10 | s12 | w1 | w11
    dsc = nc.dram_tensor("dsc", [nzz2, 128], BF16, kind="Internal")

    with TileContext(nc) as tc:
        with tc.tile_pool(name="sbuf", bufs=1) as pool, tc.tile_pool(
            name="psum", bufs=1, space="PSUM"
        ) as ppool:
            xz_sb = [pool.tile([128, nh], BF16, name=f"xzs{k}") for k in range(2)]
            xe_sb = [pool.tile([128, nh], BF16, name=f"xes{k}") for k in range(2)]
            xg_sb = pool.tile([128, ncol], F32)
            t_sb = pool.tile([128, NE], F32)
            id_sb = pool.tile([128, 128], BF16)
            d_sb = [pool.tile([128, nh], BF16, name=f"ds{k}") for k in range(2)]
            u_sb = [pool.tile([128, nh // 2], BF16, name=f"us{k}") for k in range(2)]
            v_sb2 = [pool.tile([128, nh // 4], BF16, name=f"vs{k}") for k in range(2)]
            rank_sb = pool.tile([128, ncol], BF16)
            r_sb = pool.tile([128, nr], BF16)
            uc_sb = pool.tile([128, NJ * 96], BF16)
            vc_sb = pool.tile([128, NJ * 48], I16)
            wc_sb = pool.tile([128, NJ * 16], I16)
            ci_sb = pool.tile([128, NJ * 16], I16)
            rx_sb = pool.tile([128, nr], F32)
            uq_sb = pool.tile([128, NJ * 96], F32)
            vq_sb = pool.tile([128, NJ * 48], F32)
            tq_sb = pool.tile([128, NJ * 16], F32)
            qf_sb = pool.tile([128, (NJ + 2) * G], F32)  # [0 | Q1..Q11 | Q12]
            u12_sb = pool.tile([128, 96], F32)
            v12_sb = pool.tile([128, 48], F32)
            t12_sb = pool.tile([128, 16], F32)
            t3_sb = pool.tile([128, NJ * 16], F32)
            dd_sb = pool.tile([128, nzz], BF16)
            r12c_sb = pool.tile([128, ncol], BF16)
            dd2_sb = pool.tile([128, nzz2], BF16)
            dt_sb = [pool.tile([128, 128], BF16, name=f"dts{k}") for k in range(5)]
            dr_sb = pool.tile([128, nzz2 * 16], BF16)
            t1c_sb = pool.tile([128, N_IN], BF16)
            t11c_sb = pool.tile([128, N_IN], BF16)
            u1_sb = pool.tile([128, N_IN * nc_], BF16)
            u11_sb = pool.tile([128, N_IN * nc_], BF16)
            g_sb = pool.tile([128, 9 * nc_], F32)
            vv_sb = pool.tile([128, 9 * nc_], BF16)
            y_sb = pool.tile([128, nc_], F32)
            yacc = ppool.tile([128, nc_], F32)
            tp_ps = [
                ppool.tile([128, 128], BF16, name=f"tps{k}") for k in range(5)
            ]

            nc.gpsimd.load_library(library_config.ap_gather)

            xz_d = [xza_d, xzb_d]
            xe_d = [xea_d, xeb_d]

            for _rep in range(repeat):
                # --- input DMAs on the two HWDGE queues --------------------
                nc.sync.dma_start(out=xe_sb[0][:, :], in_=xe_d[0][:, :])
                nc.scalar.dma_start(out=xz_sb[0][:, :], in_=xz_d[0][:, :])
                nc.sync.dma_start(out=xe_sb[1][:, :], in_=xe_d[1][:, :])
                nc.scalar.dma_start(out=xz_sb[1][:, :], in_=xz_d[1][:, :])
                nc.sync.dma_start(out=xg_sb[:, :], in_=xg_d[:, :])
                nc.scalar.dma_start(out=id_sb[:, :], in_=id_d[:, :])
                nc.sync.dma_start(out=t1c_sb[:, :], in_=t1c_d[:, :])
                nc.scalar.dma_start(out=t11c_sb[:, :], in_=t11c_d[:, :])
                nc.sync.dma_start(out=t_sb[:, 0 : NE // 2], in_=t_d[:, 0 : NE // 2])
                nc.scalar.dma_start(out=t_sb[:, NE // 2 :], in_=t_d[:, NE // 2 :])

                # --- D + rank (weight-1 j-tree), per i-chunk ---------------
                for k in range(2):
                    xz, xe, d = xz_sb[k], xe_sb[k], d_sb[k]
                    u, v = u_sb[k], v_sb2[k]
                    nc.vector.tensor_tensor(
                        out=d[:, 0 : nh // 2], in0=xz[:, 0 : nh // 2],
                        in1=xe[:, 0 : nh // 2], op=mybir.AluOpType.is_ge,
                    )
                    nc.vector.tensor_tensor(
                        out=d[:, nh // 2 :], in0=xz[:, nh // 2 :],
                        in1=xe[:, nh // 2 :], op=mybir.AluOpType.is_ge,
                    )
                    # u = D[j] + D[j+6] ; v = u[j'] + u[j'+3]
                    nc.vector.tensor_tensor(
                        out=u[:, :], in0=d[:, 0 : nh // 2], in1=d[:, nh // 2 :],
                        op=add,
                    )
                    nc.vector.tensor_tensor(
                        out=v[:, :], in0=u[:, 0 : nh // 4], in1=u[:, nh // 4 :],
                        op=add,
                    )
                    # rank-half = v0 + v1 + v2  (96 cols: (i-half, g))
                    rh = rank_sb[:, k * 96 : (k + 1) * 96]
                    nc.vector.tensor_tensor(
                        out=rh, in0=v[:, 0:96], in1=v[:, 96:192], op=add
                    )
                    nc.vector.tensor_tensor(
                        out=rh, in0=rh, in1=v[:, 192:288], op=add
                    )

                # --- R-stack: R_j = [rank <= j], j = 1..11 -----------------
                for j in range(1, NJ + 1):
                    nc.vector.tensor_scalar(
                        out=r_sb[:, (j - 1) * ncol : j * ncol],
                        in0=rank_sb[:, :],
                        scalar1=float(j),
                        scalar2=None,
                        op0=mybir.AluOpType.is_le,
                    )

                rv = r_sb[:, :].rearrange("p (j z) -> p j z", j=NJ)
                ucv = uc_sb[:, :].rearrange("p (j z) -> p j z", j=NJ)
                vcv = vc_sb[:, :].rearrange("p (j z) -> p j z", j=NJ)
                wcv = wc_sb[:, :].rearrange("p (j z) -> p j z", j=NJ)
                civ = ci_sb[:, :].rearrange("p (j z) -> p j z", j=NJ)

                # --- c_j: 2^i radix tree over i ----------------------------
                nc.vector.scalar_tensor_tensor(
                    out=ucv[:, :, :], in0=rv[:, :, 96:192], scalar=64.0,
                    in1=rv[:, :, 0:96], op0=mult, op1=add,
                )
                nc.vector.scalar_tensor_tensor(
                    out=vcv[:, :, :], in0=ucv[:, :, 48:96], scalar=8.0,
                    in1=ucv[:, :, 0:48], op0=mult, op1=add,
                )
                nc.vector.scalar_tensor_tensor(
                    out=wcv[:, :, :], in0=vcv[:, :, 16:32], scalar=2.0,
                    in1=vcv[:, :, 0:16], op0=mult, op1=add,
                )
                nc.vector.scalar_tensor_tensor(
                    out=civ[:, :, :], in0=vcv[:, :, 32:48], scalar=4.0,
                    in1=wcv[:, :, :], op0=mult, op1=add,
                )

                # --- gather: 3 ap_gather calls over j-blocks 1..9 ----------
                # (j = 2..10; j = 1 and 11 have only 12 possible table rows
                # each and are handled as one-hot x T1c / T11c products)
                jsp = [(1, 5), (5, 9), (9, 10)]
                for j0, j1 in jsp:
                    nidx = (j1 - j0) * nc_
                    nc.gpsimd.ap_gather(
                        out_ap=g_sb[:, (j0 - 1) * nc_ : (j1 - 1) * nc_],
                        in_ap=t_sb[:, :],
                        idxs_ap=ci_sb[:, j0 * G : j1 * G],
                        channels=128,
                        num_elems=NE,
                        d=1,
                        num_idxs=nidx,
                    )

                # --- Q_j: x-weighted tree in f32 ---------------------------
                xgb = xg_sb[:, :].rearrange("p (o z) -> p o z", o=1).to_broadcast(
                    [128, NJ, ncol]
                )
                rxv = rx_sb[:, :].rearrange("p (j z) -> p j z", j=NJ)
                nc.vector.tensor_tensor(out=rxv[:, :, :], in0=rv, in1=xgb, op=mult)
                uqv = uq_sb[:, :].rearrange("p (j z) -> p j z", j=NJ)
                vqv = vq_sb[:, :].rearrange("p (j z) -> p j z", j=NJ)
                tqv = tq_sb[:, :].rearrange("p (j z) -> p j z", j=NJ)
                nc.vector.tensor_tensor(
                    out=uqv[:, :, :], in0=rxv[:, :, 0:96], in1=rxv[:, :, 96:192],
                    op=add,
                )
                nc.vector.tensor_tensor(
                    out=vqv[:, :, :], in0=uqv[:, :, 0:48], in1=uqv[:, :, 48:96],
                    op=add,
                )
                nc.vector.tensor_tensor(
                    out=tqv[:, :, :], in0=vqv[:, :, 0:16], in1=vqv[:, :, 16:32],
                    op=add,
                )
                # Qf blocks: [0:16]=0, [16:192]=Q_1..11, [192:208]=Q12
                nc.vector.memset(qf_sb[:, 0:G], 0.0)
                nc.vector.tensor_tensor(
                    out=qf_sb[:, G : (NJ + 1) * G].rearrange(
                        "p (j z) -> p j z", j=NJ
                    ),
                    in0=tqv[:, :, :], in1=vqv[:, :, 32:48], op=add,
                )
                # Q12 = sum_i x_i
                nc.vector.tensor_tensor(
                    out=u12_sb[:, :], in0=xg_sb[:, 0:96], in1=xg_sb[:, 96:192],
                    op=add,
                )
                nc.vector.tensor_tensor(
                    out=v12_sb[:, :], in0=u12_sb[:, 0:48], in1=u12_sb[:, 48:96],
                    op=add,
                )
                nc.vector.tensor_tensor(
                    out=t12_sb[:, :], in0=v12_sb[:, 0:16], in1=v12_sb[:, 16:32],
                    op=add,
                )
                nc.vector.tensor_tensor(
                    out=qf_sb[:, (NJ + 1) * G : (NJ + 2) * G],
                    in0=t12_sb[:, :], in1=v12_sb[:, 32:48], op=add,
                )

                # --- d_j = 2Q_j - Q_{j-1} - Q_{j+1}; s12 = Q12 - Q11 -------
                nc.vector.scalar_tensor_tensor(
                    out=t3_sb[:, :], in0=qf_sb[:, G : (NJ + 1) * G], scalar=2.0,
                    in1=qf_sb[:, 0 : NJ * G], op0=mult, op1=sub,
                )
                nc.vector.tensor_tensor(
                    out=dd_sb[:, 0 : NJ * G], in0=t3_sb[:, :],
                    in1=qf_sb[:, 2 * G : (NJ + 2) * G], op=sub,
                )
                nc.vector.tensor_tensor(
                    out=dd_sb[:, NJ * G : (NJ + 1) * G],
                    in0=qf_sb[:, (NJ + 1) * G : (NJ + 2) * G],
                    in1=qf_sb[:, NJ * G : (NJ + 1) * G], op=sub,
                )

                # --- dd2: [d2..d10 | s12 | w1 | w11]  (544 cols bf16) ------
                nc.vector.tensor_copy(
                    out=dd2_sb[:, 0 : 9 * G], in_=dd_sb[:, G : 10 * G]
                )
                nc.vector.tensor_copy(
                    out=dd2_sb[:, 9 * G : 10 * G],
                    in_=dd_sb[:, NJ * G : (NJ + 1) * G],
                )
                # w1 = R_1 (argmax one-hot) * d_1, per (a, g)
                nc.vector.tensor_tensor(
                    out=dd2_sb[:, 10 * G : 10 * G + N_IN * G].rearrange(
                        "p (a z) -> p a z", a=N_IN
                    ),
                    in0=r_sb[:, 0:ncol].rearrange("p (a z) -> p a z", a=N_IN),
                    in1=dd_sb[:, 0:G].rearrange("p (o z) -> p o z", o=1)
                    .to_broadcast([128, N_IN, G]),
                    op=mult,
                )
                # w11 = [rank = 12] (argmin one-hot) * d_11
                nc.vector.tensor_scalar(
                    out=r12c_sb[:, :], in0=rank_sb[:, :], scalar1=12.0,
                    scalar2=None, op0=mybir.AluOpType.is_ge,
                )
                nc.vector.tensor_tensor(
                    out=dd2_sb[:, 10 * G + N_IN * G :].rearrange(
                        "p (a z) -> p a z", a=N_IN
                    ),
                    in0=r12c_sb[:, :].rearrange("p (a z) -> p a z", a=N_IN),
                    in1=dd_sb[:, 10 * G : NJ * G].rearrange("p (o z) -> p o z", o=1)
                    .to_broadcast([128, N_IN, G]),
                    op=mult,
                )

                # --- o-replication of dd2: PE transpose -> DRAM -> 16 reads
                tsl = [(0, 128), (128, 256), (256, 384), (384, 512), (512, 544)]
                for k, (o0, o1) in enumerate(tsl):
                    w = o1 - o0
                    nc.tensor.transpose(
                        out=tp_ps[k][0:w, :],
                        in_=dd2_sb[:, o0:o1],
                        identity=id_sb[:, :],
                    )
                    nc.vector.tensor_copy(
                        out=dt_sb[k][0:w, :], in_=tp_ps[k][0:w, :]
                    )
                    eng = nc.sync if k % 2 == 0 else nc.scalar
                    eng.dma_start(out=dsc[o0:o1, :], in_=dt_sb[k][0:w, :])
                dscv = dsc[:, :].rearrange("z (c q) -> c z q", q=16)
                drv = dr_sb[:, :].rearrange("p (z q) -> p z q", q=16)
                for o in range(16):
                    eng = nc.sync if o % 2 == 0 else nc.scalar
                    eng.dma_start(out=drv[o::16, :, :], in_=dscv[:, :, :])

                # --- s12 + one-hot j=1/j=11 contributions on PE ------------
                nc.tensor.matmul(
                    out=yacc[:, :],
                    lhsT=id_sb[:, :],
                    rhs=dr_sb[:, 9 * nc_ : 10 * nc_],
                    start=True,
                    stop=False,
                )
                for (usb, tcs, zb) in (
                    (u1_sb, t1c_sb, 10 * G),
                    (u11_sb, t11c_sb, 10 * G + N_IN * G),
                ):
                    nc.vector.tensor_tensor(
                        out=usb[:, :].rearrange("p (a z) -> p a z", a=N_IN),
                        in0=dr_sb[:, zb * 16 : (zb + N_IN * G) * 16].rearrange(
                            "p (a z) -> p a z", a=N_IN
                        ),
                        in1=tcs[:, :].rearrange("p (a z) -> p a z", z=1)
                        .to_broadcast([128, N_IN, nc_]),
                        op=mult,
                    )
                    for a in range(N_IN):
                        nc.tensor.matmul(
                            out=yacc[:, :],
                            lhsT=id_sb[:, :],
                            rhs=usb[:, a * nc_ : (a + 1) * nc_],
                            start=False,
                            stop=False,
                        )

                # --- combine per gathered chunk: V = G * dr; PE reduce -----
                for j0, j1 in jsp:
                    b0, b1 = j0 - 1, j1 - 1
                    nc.vector.tensor_tensor(
                        out=vv_sb[:, b0 * nc_ : b1 * nc_],
                        in0=g_sb[:, b0 * nc_ : b1 * nc_],
                        in1=dr_sb[:, b0 * nc_ : b1 * nc_], op=mult,
                    )
                    for jb in range(b0, b1):
                        nc.tensor.matmul(
                            out=yacc[:, :],
                            lhsT=id_sb[:, :],
                            rhs=vv_sb[:, jb * nc_ : (jb + 1) * nc_],
                            start=False,
                            stop=(jb == 8),
                        )

                nc.vector.tensor_copy(out=y_sb[:, :], in_=yacc[:, :])
                nc.sync.dma_start(out=y_d[:, :], in_=y_sb[:, :])

    nc.compile()
    return nc


_NC_CACHE: dict[tuple, bass.Bass] = {}


def _get_nc(m_core: int, repeat: int = 1) -> bass.Bass:
    key = (m_core, repeat)
    if key not in _NC_CACHE:
        _NC_CACHE[key] = build_bass(m_core, repeat)
    return _NC_CACHE[key]


def _prep_core_inputs(x_shard: np.ndarray, t_rep: np.ndarray) -> dict:
    """Host-side input prep (layout/dtype transforms only).
    Row m = c*256 + g*16 + q lives on partition p = 16c+q; col = i*G + g."""
    m_core = x_shard.shape[0]
    G = m_core // 128
    x5 = x_shard.reshape(8, G, 16, N_IN).astype(np.float32)
    xb5 = x5.astype(BF)
    xe5 = xb5.transpose(0, 2, 3, 1)  # [c, q, i, g]
    xj = xe5
    dn = np.nextafter(xj, np.array(-np.inf, BF))
    xz = np.empty((8, 16, N_IN, N_IN, G), BF)  # [c, q, j, i, g]
    xz[:] = xj[:, :, :, None, :]
    ii = np.arange(N_IN)
    lower = ii[None, :] > ii[:, None]  # [j, i]: i < j
    xz[:, :, lower] = np.broadcast_to(dn[:, :, :, None, :], xz.shape)[:, :, lower]
    hi = N_IN // 2
    xza = xz[:, :, :, :hi, :].reshape(128, -1)
    xzb = xz[:, :, :, hi:, :].reshape(128, -1)
    xe12 = np.broadcast_to(xe5[:, :, None, :, :], (8, 16, N_IN, N_IN, G))
    xea = np.ascontiguousarray(xe12[:, :, :, :hi, :]).reshape(128, -1)
    xeb = np.ascontiguousarray(xe12[:, :, :, hi:, :]).reshape(128, -1)
    # xg[16c+q, i*G+g] = x5[c, g, q, i] in f32 (Q-tree weights)
    xg = np.ascontiguousarray(x5.transpose(0, 2, 3, 1).reshape(128, -1))
    ident = np.eye(128, dtype=BF)
    pw = 2 ** np.arange(N_IN)
    return {
        "t": t_rep,
        "t1c": np.ascontiguousarray(t_rep[:, pw].astype(BF)),
        "t11c": np.ascontiguousarray(t_rep[:, (NE - 1) - pw].astype(BF)),
        "xza": np.ascontiguousarray(xza),
        "xzb": np.ascontiguousarray(xzb),
        "xea": np.ascontiguousarray(xea),
        "xeb": np.ascontiguousarray(xeb),
        "xg": xg,
        "ident": ident,
    }


def _post_core_output(y_dev: np.ndarray, m_core: int) -> np.ndarray:
    # y_dev [128, G*16]: [16c+o, g*16+q] -> y[c*256+g*16+q, o]
    G = m_core // 128
    y = y_dev.reshape(8, 16, G, 16)  # [c, o, g, q]
    y = y.transpose(0, 2, 3, 1)  # [c, g, q, o]
    return np.ascontiguousarray(y.reshape(m_core, 16))


def kernel(inputs: np.ndarray, fm_vars: np.ndarray, _repeat: int = 1) -> np.ndarray:
    inputs = np.ascontiguousarray(np.asarray(inputs, dtype=np.float32))
    fm_vars = np.asarray(fm_vars, dtype=np.float32)
    assert inputs.shape == (M_FULL, N_IN), inputs.shape
    table = _build_table(fm_vars)  # [4096, 16]
    t_rep = np.ascontiguousarray(np.tile(table.T, (8, 1)))  # [128, 4096]

    nc = _get_nc(M_CORE, _repeat)
    shards = inputs.reshape(N_CORES, M_CORE, N_IN)
    in_maps = [_prep_core_inputs(shards[c], t_rep) for c in range(N_CORES)]
    res = run_bass_kernel_spmd(nc, in_maps, list(range(N_CORES)))
    out = np.concatenate(
        [_post_core_output(r["y"], M_CORE) for r in res.results], axis=0
    )
    return out.astype(np.float32)


# revision 9
# speedup vs baseline: 1.1060x; 1.1060x over previous
"""Trainium2 Bass kernel for nn_Choquet_integral (N_IN=12, N_OUT=16, M=16384).

Math (per input row x[0:12], fuzzy-measure table FM[4095, 16]):
    reference: sort x descending -> s, diffs_j = s_j - s_{j+1} (s_12 = 0),
    cumulative-set index c_j = sum_{t<=j} 2^{sortInd_t} - 1,
    out = sum_j diffs_j * FM[c_j].

v5: ranked-prefix (c_j) formulation -- 11 table lookups per row instead of
v3's 24 (A/B Abel form).  With T[v] = FM[v-1], T[0] = 0 ([4096, 16]):

    rank_i = #(j : x_j >=~ x_i)     (total order via 1-ulp bf16 nudge, i<j)
    R_j[i] = [rank_i <= j]          (top-j membership, j = 1..11)
    c_j    = sum_i 2^i R_j[i]       (top-j set mask; c_12 = 4095, T = 1)
    Q_j    = sum_i x_i R_j[i]       (sum of top-j values; Q_0 = 0)
    d_j    = 2Q_j - Q_{j-1} - Q_{j+1} = s_j - s_{j+1}
    y      = sum_{j=1..11} d_j T[c_j] + (Q_12 - Q_11)

Per core (2048 rows), layout partition p = 16c+q <-> row c*256+g*16+q:
  1. D = is_ge comparisons (v3's comparand tiles, 2 i-chunks x [128,1152]).
  2. rank via a weight-1 pairwise tree over the j dim (4 ops/chunk).
  3. R-stack [128, 11*192] via 11 tensor_scalar is_le ops.
  4. c_j via the 2^i radix tree over i (4 strided stt ops, int16 exact)
     -> c_int [128, 11*16] which IS the ap_gather index tile (col (j,g),
     partition-slot q: gather's 16-partition index sharing o-replicates).
  5. Q_j via x-weighted tree in f32 (bf16 would lose ~0.05 abs on Q~6).
  6. d -> dd [128, 192] bf16 (blocks d_1..d_11, s12).
  7. dd must reach the combine o-replicated: PE-transpose dd -> [192, 128]
     (2 matmul transposes), DMA to a DRAM scratch [192, 128], then 16
     partition-strided reads dr[16c+o, (jb*16+g)*16+q] <- dsc[z, c*16+q]
     (descriptors are 32B q-runs, contiguous both sides).
  8. 3 ap_gather calls (j-blocks 5+5+1; small last chunk shortens the
     post-gather tail) from the f32 column table
     t[16c+o] = T[:, o]: 2816 idx/DSP-core total vs v3's 6144 -- the Q7
     request loop is ~27ns/idx and dominates the kernel.
  9. combine V = G * dr (bf16), PE: 12 identity matmuls (11 gathered j's
     + the s12 block) accumulate yacc [128, 256]; copy out.
"""

import numpy as np
import ml_dtypes

import concourse.bacc as bacc
import concourse.bass as bass
import concourse.mybir as mybir
from concourse import library_config
from concourse.bass_utils import run_bass_kernel_spmd
from concourse.tile import TileContext

N_IN = 12
N_OUT = 16
N_VARS = 2**N_IN - 2  # 4094
M_FULL = 16384
N_CORES = 8
M_CORE = M_FULL // N_CORES  # 2048
NE = 2**N_IN  # 4096 table entries
NJ = 11  # gathered j-levels
F32 = mybir.dt.float32
BF16 = mybir.dt.bfloat16
I16 = mybir.dt.int16
BF = ml_dtypes.bfloat16


def _lattice_levels(n_in):
    levels = []
    for k in range(2, n_in):
        nodes = [s for s in range(1, 2**n_in - 1) if bin(s).count("1") == k]
        children = [
            [(s - (1 << b)) - 1 for b in range(n_in) if (s >> b) & 1] for s in nodes
        ]
        levels.append((np.array(nodes) - 1, np.array(children)))
    return levels


_LEVELS = _lattice_levels(N_IN)
_SINGLETONS = np.array([2**i - 1 for i in range(N_IN)])


def _build_table(fm_vars: np.ndarray) -> np.ndarray:
    """T_ext [4096, 16]: T_ext[0] = 0, T_ext[v] = FM[v-1]."""
    av = np.abs(fm_vars.astype(np.float32))
    FM = np.zeros((N_VARS, N_OUT), np.float32)
    FM[_SINGLETONS] = av[_SINGLETONS]
    for nodes_idx, children_idx in _LEVELS:
        mx = FM[children_idx].max(axis=1)
        FM[nodes_idx] = mx + av[nodes_idx]
    FM = np.concatenate([FM, np.ones((1, N_OUT), np.float32)], axis=0)
    FM = np.minimum(FM, np.float32(1.0))
    return np.concatenate([np.zeros((1, N_OUT), np.float32), FM], axis=0)


def build_bass(m_core: int = M_CORE, repeat: int = 1) -> bass.Bass:
    assert m_core % 2048 == 0
    G = m_core // 128  # 16 row-groups per partition
    n = N_IN
    ncol = n * G  # 192 (i-major: col = i*G + g)
    nj = n * ncol  # 2304
    nh = nj // 2  # 1152 per i-chunk (j 12 x i-half 6 x g 16)
    nr = NJ * ncol  # 2112 R-stack cols (j 1..11, i, g)
    nzz = (NJ + 1) * G  # 192 dd cols (jb 0..11, g)
    nc_ = G * 16  # 256 output cols (g*16+q)
    ngt = NJ * nc_  # 2816 gathered cols
    nc = bacc.Bacc()

    add = mybir.AluOpType.add
    mult = mybir.AluOpType.mult
    sub = mybir.AluOpType.subtract

    xza_d = nc.declare_dram_parameter("xza", [128, nh], BF16, isOutput=False)
    xzb_d = nc.declare_dram_parameter("xzb", [128, nh], BF16, isOutput=False)
    xea_d = nc.declare_dram_parameter("xea", [128, nh], BF16, isOutput=False)
    xeb_d = nc.declare_dram_parameter("xeb", [128, nh], BF16, isOutput=False)
    xg_d = nc.declare_dram_parameter("xg", [128, ncol], F32, isOutput=False)
    t_d = nc.declare_dram_parameter("t", [128, NE], F32, isOutput=False)
    id_d = nc.declare_dram_parameter("ident", [128, 128], BF16, isOutput=False)
    y_d = nc.declare_dram_parameter("y", [128, nc_], F32, isOutput=True)

    dsc = nc.dram_tensor("dsc", [nzz, 128], BF16, kind="Internal")

    with TileContext(nc) as tc:
        with tc.tile_pool(name="sbuf", bufs=1) as pool, tc.tile_pool(
            name="psum", bufs=1, space="PSUM"
        ) as ppool:
            xz_sb = [pool.tile([128, nh], BF16, name=f"xzs{k}") for k in range(2)]
            xe_sb = [pool.tile([128, nh], BF16, name=f"xes{k}") for k in range(2)]
            xg_sb = pool.tile([128, ncol], F32)
            t_sb = pool.tile([128, NE], F32)
            id_sb = pool.tile([128, 128], BF16)
            d_sb = [pool.tile([128, nh], BF16, name=f"ds{k}") for k in range(2)]
            u_sb = [pool.tile([128, nh // 2], BF16, name=f"us{k}") for k in range(2)]
            v_sb2 = [pool.tile([128, nh // 4], BF16, name=f"vs{k}") for k in range(2)]
            rank_sb = pool.tile([128, ncol], BF16)
            r_sb = pool.tile([128, nr], BF16)
            uc_sb = pool.tile([128, NJ * 96], BF16)
            vc_sb = pool.tile([128, NJ * 48], I16)
            wc_sb = pool.tile([128, NJ * 16], I16)
            ci_sb = pool.tile([128, NJ * 16], I16)
            rx_sb = pool.tile([128, nr], F32)
            uq_sb = pool.tile([128, NJ * 96], F32)
            vq_sb = pool.tile([128, NJ * 48], F32)
            tq_sb = pool.tile([128, NJ * 16], F32)
            qf_sb = pool.tile([128, (NJ + 2) * G], F32)  # [0 | Q1..Q11 | Q12]
            u12_sb = pool.tile([128, 96], F32)
            v12_sb = pool.tile([128, 48], F32)
            t12_sb = pool.tile([128, 16], F32)
            t3_sb = pool.tile([128, NJ * 16], F32)
            dd_sb = pool.tile([128, nzz], BF16)
            dt_sb = [pool.tile([96, 128], BF16, name=f"dts{k}") for k in range(2)]
            dr_sb = pool.tile([128, nzz * 16], BF16)
            g_sb = pool.tile([128, ngt], F32)
            vv_sb = pool.tile([128, ngt], BF16)
            y_sb = pool.tile([128, nc_], F32)
            yacc = ppool.tile([128, nc_], F32)
            tp_ps = [
                ppool.tile([96, 128], BF16, name=f"tps{k}") for k in range(2)
            ]

            nc.gpsimd.load_library(library_config.ap_gather)

            xz_d = [xza_d, xzb_d]
            xe_d = [xea_d, xeb_d]

            for _rep in range(repeat):
                # --- input DMAs on the two HWDGE queues --------------------
                nc.sync.dma_start(out=xe_sb[0][:, :], in_=xe_d[0][:, :])
                nc.scalar.dma_start(out=xz_sb[0][:, :], in_=xz_d[0][:, :])
                nc.sync.dma_start(out=xe_sb[1][:, :], in_=xe_d[1][:, :])
                nc.scalar.dma_start(out=xz_sb[1][:, :], in_=xz_d[1][:, :])
                nc.sync.dma_start(out=xg_sb[:, :], in_=xg_d[:, :])
                nc.scalar.dma_start(out=id_sb[:, :], in_=id_d[:, :])
                nc.sync.dma_start(out=t_sb[:, 0 : NE // 2], in_=t_d[:, 0 : NE // 2])
                nc.scalar.dma_start(out=t_sb[:, NE // 2 :], in_=t_d[:, NE // 2 :])

                # --- D + rank (weight-1 j-tree), per i-chunk ---------------
                for k in range(2):
                    xz, xe, d = xz_sb[k], xe_sb[k], d_sb[k]
                    u, v = u_sb[k], v_sb2[k]
                    nc.vector.tensor_tensor(
                        out=d[:, 0 : nh // 2], in0=xz[:, 0 : nh // 2],
                        in1=xe[:, 0 : nh // 2], op=mybir.AluOpType.is_ge,
                    )
                    nc.vector.tensor_tensor(
                        out=d[:, nh // 2 :], in0=xz[:, nh // 2 :],
                        in1=xe[:, nh // 2 :], op=mybir.AluOpType.is_ge,
                    )
                    # u = D[j] + D[j+6] ; v = u[j'] + u[j'+3]
                    nc.vector.tensor_tensor(
                        out=u[:, :], in0=d[:, 0 : nh // 2], in1=d[:, nh // 2 :],
                        op=add,
                    )
                    nc.vector.tensor_tensor(
                        out=v[:, :], in0=u[:, 0 : nh // 4], in1=u[:, nh // 4 :],
                        op=add,
                    )
                    # rank-half = v0 + v1 + v2  (96 cols: (i-half, g))
                    rh = rank_sb[:, k * 96 : (k + 1) * 96]
                    nc.vector.tensor_tensor(
                        out=rh, in0=v[:, 0:96], in1=v[:, 96:192], op=add
                    )
                    nc.vector.tensor_tensor(
                        out=rh, in0=rh, in1=v[:, 192:288], op=add
                    )

                # --- R-stack: R_j = [rank <= j], j = 1..11 -----------------
                for j in range(1, NJ + 1):
                    nc.vector.tensor_scalar(
                        out=r_sb[:, (j - 1) * ncol : j * ncol],
                        in0=rank_sb[:, :],
                        scalar1=float(j),
                        scalar2=None,
                        op0=mybir.AluOpType.is_le,
                    )

                rv = r_sb[:, :].rearrange("p (j z) -> p j z", j=NJ)
                ucv = uc_sb[:, :].rearrange("p (j z) -> p j z", j=NJ)
                vcv = vc_sb[:, :].rearrange("p (j z) -> p j z", j=NJ)
                wcv = wc_sb[:, :].rearrange("p (j z) -> p j z", j=NJ)
                civ = ci_sb[:, :].rearrange("p (j z) -> p j z", j=NJ)

                # --- c_j: 2^i radix tree over i ----------------------------
                nc.vector.scalar_tensor_tensor(
                    out=ucv[:, :, :], in0=rv[:, :, 96:192], scalar=64.0,
                    in1=rv[:, :, 0:96], op0=mult, op1=add,
                )
                nc.vector.scalar_tensor_tensor(
                    out=vcv[:, :, :], in0=ucv[:, :, 48:96], scalar=8.0,
                    in1=ucv[:, :, 0:48], op0=mult, op1=add,
                )
                nc.vector.scalar_tensor_tensor(
                    out=wcv[:, :, :], in0=vcv[:, :, 16:32], scalar=2.0,
                    in1=vcv[:, :, 0:16], op0=mult, op1=add,
                )
                nc.vector.scalar_tensor_tensor(
                    out=civ[:, :, :], in0=vcv[:, :, 32:48], scalar=4.0,
                    in1=wcv[:, :, :], op0=mult, op1=add,
                )

                # --- gather: 3 ap_gather calls over j-blocks ---------------
                jsp = [(0, 5), (5, 10)]
                for j0, j1 in jsp:
                    nidx = (j1 - j0) * nc_
                    nc.gpsimd.ap_gather(
                        out_ap=g_sb[:, j0 * nc_ : j1 * nc_],
                        in_ap=t_sb[:, :],
                        idxs_ap=ci_sb[:, j0 * G : j1 * G],
                        channels=128,
                        num_elems=NE,
                        d=1,
                        num_idxs=nidx,
                    )
                # j-block 10 in two 128-idx g-halves for a minimal tail
                for h in range(2):
                    nc.gpsimd.ap_gather(
                        out_ap=g_sb[:, 10 * nc_ + h * 128 : 10 * nc_ + (h + 1) * 128],
                        in_ap=t_sb[:, :],
                        idxs_ap=ci_sb[:, 10 * G + h * 8 : 10 * G + (h + 1) * 8],
                        channels=128,
                        num_elems=NE,
                        d=1,
                        num_idxs=128,
                    )

                # --- Q_j: x-weighted tree in f32 ---------------------------
                xgb = xg_sb[:, :].rearrange("p (o z) -> p o z", o=1).to_broadcast(
                    [128, NJ, ncol]
                )
                rxv = rx_sb[:, :].rearrange("p (j z) -> p j z", j=NJ)
                nc.vector.tensor_tensor(out=rxv[:, :, :], in0=rv, in1=xgb, op=mult)
                uqv = uq_sb[:, :].rearrange("p (j z) -> p j z", j=NJ)
                vqv = vq_sb[:, :].rearrange("p (j z) -> p j z", j=NJ)
                tqv = tq_sb[:, :].rearrange("p (j z) -> p j z", j=NJ)
                nc.vector.tensor_tensor(
                    out=uqv[:, :, :], in0=rxv[:, :, 0:96], in1=rxv[:, :, 96:192],
                    op=add,
                )
                nc.vector.tensor_tensor(
                    out=vqv[:, :, :], in0=uqv[:, :, 0:48], in1=uqv[:, :, 48:96],
                    op=add,
                )
                nc.vector.tensor_tensor(
                    out=tqv[:, :, :], in0=vqv[:, :, 0:16], in1=vqv[:, :, 16:32],
                    op=add,
                )
                # Qf blocks: [0:16]=0, [16:192]=Q_1..11, [192:208]=Q12
                nc.vector.memset(qf_sb[:, 0:G], 0.0)
                nc.vector.tensor_tensor(
                    out=qf_sb[:, G : (NJ + 1) * G].rearrange(
                        "p (j z) -> p j z", j=NJ
                    ),
                    in0=tqv[:, :, :], in1=vqv[:, :, 32:48], op=add,
                )
                # Q12 = sum_i x_i
                nc.vector.tensor_tensor(
                    out=u12_sb[:, :], in0=xg_sb[:, 0:96], in1=xg_sb[:, 96:192],
                    op=add,
                )
                nc.vector.tensor_tensor(
                    out=v12_sb[:, :], in0=u12_sb[:, 0:48], in1=u12_sb[:, 48:96],
                    op=add,
                )
                nc.vector.tensor_tensor(
                    out=t12_sb[:, :], in0=v12_sb[:, 0:16], in1=v12_sb[:, 16:32],
                    op=add,
                )
                nc.vector.tensor_tensor(
                    out=qf_sb[:, (NJ + 1) * G : (NJ + 2) * G],
                    in0=t12_sb[:, :], in1=v12_sb[:, 32:48], op=add,
                )

                # --- d_j = 2Q_j - Q_{j-1} - Q_{j+1}; s12 = Q12 - Q11 -------
                nc.vector.scalar_tensor_tensor(
                    out=t3_sb[:, :], in0=qf_sb[:, G : (NJ + 1) * G], scalar=2.0,
                    in1=qf_sb[:, 0 : NJ * G], op0=mult, op1=sub,
                )
                nc.vector.tensor_tensor(
                    out=dd_sb[:, 0 : NJ * G], in0=t3_sb[:, :],
                    in1=qf_sb[:, 2 * G : (NJ + 2) * G], op=sub,
                )
                nc.vector.tensor_tensor(
                    out=dd_sb[:, NJ * G : (NJ + 1) * G],
                    in0=qf_sb[:, (NJ + 1) * G : (NJ + 2) * G],
                    in1=qf_sb[:, NJ * G : (NJ + 1) * G], op=sub,
                )

                # --- o-replication of dd: PE transpose -> DRAM -> 16 reads -
                for k in range(2):
                    nc.tensor.transpose(
                        out=tp_ps[k][:, :],
                        in_=dd_sb[:, k * 96 : (k + 1) * 96],
                        identity=id_sb[:, :],
                    )
                    nc.vector.tensor_copy(out=dt_sb[k][:, :], in_=tp_ps[k][:, :])
                    nc.sync.dma_start(
                        out=dsc[k * 96 : (k + 1) * 96, :], in_=dt_sb[k][:, :]
                    )
                dscv = dsc[:, :].rearrange("z (c q) -> c z q", q=16)
                drv = dr_sb[:, :].rearrange("p (z q) -> p z q", q=16)
                for o in range(16):
                    eng = nc.sync if o % 2 == 0 else nc.scalar
                    eng.dma_start(out=drv[o::16, :, :], in_=dscv[:, :, :])

                # s12 block first: PE starts the yacc accumulation as soon
                # as dr lands, off the post-gather critical path
                nc.tensor.matmul(
                    out=yacc[:, :],
                    lhsT=id_sb[:, :],
                    rhs=dr_sb[:, NJ * nc_ : (NJ + 1) * nc_],
                    start=True,
                    stop=False,
                )

                # --- combine per j-chunk: V = G * dr; PE reduce ------------
                for j0, j1 in jsp:
                    nc.vector.tensor_tensor(
                        out=vv_sb[:, j0 * nc_ : j1 * nc_],
                        in0=g_sb[:, j0 * nc_ : j1 * nc_],
                        in1=dr_sb[:, j0 * nc_ : j1 * nc_], op=mult,
                    )
                    for jb in range(j0, j1):
                        nc.tensor.matmul(
                            out=yacc[:, :],
                            lhsT=id_sb[:, :],
                            rhs=vv_sb[:, jb * nc_ : (jb + 1) * nc_],
                            start=False,
                            stop=False,
                        )
                # j-block 10 halves: col-half matmuls close the accumulation
                for h in range(2):
                    cs = slice(10 * nc_ + h * 128, 10 * nc_ + (h + 1) * 128)
                    nc.vector.tensor_tensor(
                        out=vv_sb[:, cs], in0=g_sb[:, cs], in1=dr_sb[:, cs],
                        op=mult,
                    )
                    nc.tensor.matmul(
                        out=yacc[:, h * 128 : (h + 1) * 128],
                        lhsT=id_sb[:, :],
                        rhs=vv_sb[:, cs],
                        start=False,
                        stop=True,
                    )

                nc.vector.tensor_copy(out=y_sb[:, :], in_=yacc[:, :])
                nc.sync.dma_start(out=y_d[:, :], in_=y_sb[:, :])

    nc.compile()
    return nc


_NC_CACHE: dict[tuple, bass.Bass] = {}


def _get_nc(m_core: int, repeat: int = 1) -> bass.Bass:
    key = (m_core, repeat)
    if key not in _NC_CACHE:
        _NC_CACHE[key] = build_bass(m_core, repeat)
    return _NC_CACHE[key]


def _prep_core_inputs(x_shard: np.ndarray, t_rep: np.ndarray) -> dict:
    """Host-side input prep (layout/dtype transforms only).
    Row m = c*256 + g*16 + q lives on partition p = 16c+q; col = i*G + g."""
    m_core = x_shard.shape[0]
    G = m_core // 128
    x5 = x_shard.reshape(8, G, 16, N_IN).astype(np.float32)
    xb5 = x5.astype(BF)
    xe5 = xb5.transpose(0, 2, 3, 1)  # [c, q, i, g]
    xj = xe5
    dn = np.nextafter(xj, np.array(-np.inf, BF))
    xz = np.empty((8, 16, N_IN, N_IN, G), BF)  # [c, q, j, i, g]
    xz[:] = xj[:, :, :, None, :]
    ii = np.arange(N_IN)
    lower = ii[None, :] > ii[:, None]  # [j, i]: i < j
    xz[:, :, lower] = np.broadcast_to(dn[:, :, :, None, :], xz.shape)[:, :, lower]
    hi = N_IN // 2
    xza = xz[:, :, :, :hi, :].reshape(128, -1)
    xzb = xz[:, :, :, hi:, :].reshape(128, -1)
    xe12 = np.broadcast_to(xe5[:, :, None, :, :], (8, 16, N_IN, N_IN, G))
    xea = np.ascontiguousarray(xe12[:, :, :, :hi, :]).reshape(128, -1)
    xeb = np.ascontiguousarray(xe12[:, :, :, hi:, :]).reshape(128, -1)
    # xg[16c+q, i*G+g] = x5[c, g, q, i] in f32 (Q-tree weights)
    xg = np.ascontiguousarray(x5.transpose(0, 2, 3, 1).reshape(128, -1))
    ident = np.eye(128, dtype=BF)
    return {
        "t": t_rep,
        "xza": np.ascontiguousarray(xza),
        "xzb": np.ascontiguousarray(xzb),
        "xea": np.ascontiguousarray(xea),
        "xeb": np.ascontiguousarray(xeb),
        "xg": xg,
        "ident": ident,
    }


def _post_core_output(y_dev: np.ndarray, m_core: int) -> np.ndarray:
    # y_dev [128, G*16]: [16c+o, g*16+q] -> y[c*256+g*16+q, o]
    G = m_core // 128
    y = y_dev.reshape(8, 16, G, 16)  # [c, o, g, q]
    y = y.transpose(0, 2, 3, 1)  # [c, g, q, o]
    return np.ascontiguousarray(y.reshape(m_core, 16))


def kernel(inputs: np.ndarray, fm_vars: np.ndarray, _repeat: int = 1) -> np.ndarray:
    inputs = np.ascontiguousarray(np.asarray(inputs, dtype=np.float32))
    fm_vars = np.asarray(fm_vars, dtype=np.float32)
    assert inputs.shape == (M_FULL, N_IN), inputs.shape
    table = _build_table(fm_vars)  # [4096, 16]
    t_rep = np.ascontiguousarray(np.tile(table.T, (8, 1)))  # [128, 4096]

    nc = _get_nc(M_CORE, _repeat)
    shards = inputs.reshape(N_CORES, M_CORE, N_IN)
    in_maps = [_prep_core_inputs(shards[c], t_rep) for c in range(N_CORES)]
    res = run_bass_kernel_spmd(nc, in_maps, list(range(N_CORES)))
    out = np.concatenate(
        [_post_core_output(r["y"], M_CORE) for r in res.results], axis=0
    )
    return out.astype(np.float32)
